# revision 8
# baseline (speedup 1.0000x reference)
"""Trainium2 Bass kernel for nn_CoordfnTopologyLayer (TOGL-style coordinate-function
topology layer).

Contract: kernel(**inputs) takes the FULL unsharded inputs (as produced by the
problem's setup_inputs) and returns the FULL output tuple (out [N,128], g1 [B,72]).

Strategy (graph/data parallel per the sharding hint):
  - Shard the N=100000 nodes across 8 NeuronCores (12500 each, zero-padded to
    12800). All parameters are tiny and replicated.
  - Device computes, per node shard:
        h   = relu(x @ Wf1 + bf1)                  [n, 64]
        v^  = h @ Wf2  (bias folded into consts)   [n, 8]
        act0 = coord functions of v = v^+bf2       [n, 72]
        out = relu(x @ Wout[:128] + act0 @ Wout[128:] + bout)
    All matmuls run feature-major (features on partitions) with fp32r.
  - The dim-1 persistence output g1 [200, 72] depends only on the B*F = 1600
    randomly selected edges, i.e. <=3200 node filtration values. It is computed
    exactly on the host in numpy (~26 MFLOP).

Math identities used on device:
  triangle:  relu(v - |t - v|) = max(min(2v - t, t), 0)
  gaussian:  exp(-((v-c0)^2 + (v-c1)^2)/(2s^2)) = exp(-(10(v-cbar))^2 + gamma),
             s=0.1, cbar=(c0+c1)/2, gamma = 25(c0+c1)^2 - 50(c0^2+c1^2)
  line:      (W0+W1)*v + b
  The 8 -> 72 row replication (each filtration feeds 9 coord funcs) is folded
  into the second MLP weight: Wf2r = tile(Wf2, 9) so one matmul produces the
  replicated pre-activation rows directly.

Row order of the on-device act0 (and the correspondingly permuted Wout rows),
padded to 96 rows so every partition-sliced op starts 32-aligned:
  rows  0:24  triangle, k-major then f  (c = k*8+f);  24:32 zero pad
  rows 32:56  line;                                   56:64 zero pad
  rows 64:88  gaussian;                               88:96 zero pad
"""

import os
import sys

import numpy as np

if "/opt/trn_rl_repo" not in sys.path:
    sys.path.insert(0, "/opt/trn_rl_repo")

# ---- problem constants (hardcoded per contract) ----
N = 100000
E = 1600000
B = 200
F = 8
FIN = 128
FOUT = 128
FH = 64
K = 3
SIGMA = 0.1

NCORES = 8
NSHARD = N // NCORES          # 12500
NPAD = 12800                  # per-core padded node count: 5 superchunks * 2560
SUPER = 2560                  # nodes per superchunk (20 blocks of 128)
NSUP = NPAD // SUPER          # 5
QB = SUPER // 128             # 20 blocks per superchunk
CHUNK = 512                   # nodes per compute chunk (psum free-dim limit)
JB = CHUNK // 128             # 4 blocks per chunk
NCHUNK = SUPER // CHUNK       # 5 chunks per superchunk

_CACHE = {}
LAST_RESULTS = None           # BassKernelResults of the most recent run


def _build_bass():
    """Build + schedule the per-core Bass/Tile program. Cached per process."""
    if "nc" in _CACHE:
        return _CACHE["nc"]

    import concourse.bass as bass
    import concourse.tile as tile
    from concourse import bacc, mybir

    f32 = mybir.dt.float32
    f32r = mybir.dt.float32r
    AF = mybir.ActivationFunctionType
    OP = mybir.AluOpType

    nc = bacc.Bacc("TRN2", target_bir_lowering=False, debug=False)

    # ---- DRAM I/O ----
    xs = nc.dram_tensor("xs", [NPAD, FIN], f32, kind="ExternalInput")
    wf1 = nc.dram_tensor("wf1", [FIN, FH], f32, kind="ExternalInput")
    bf1 = nc.dram_tensor("bf1", [FH, 1], f32, kind="ExternalInput")
    wf2r = nc.dram_tensor("wf2r", [FH, 96], f32, kind="ExternalInput")
    wout1 = nc.dram_tensor("wout1", [FIN, FOUT], f32, kind="ExternalInput")
    wout2p = nc.dram_tensor("wout2p", [96, FOUT], f32, kind="ExternalInput")
    boutc = nc.dram_tensor("boutc", [FOUT, 1], f32, kind="ExternalInput")
    a96 = nc.dram_tensor("a96", [96, 1], f32, kind="ExternalInput")
    b96 = nc.dram_tensor("b96", [96, 1], f32, kind="ExternalInput")
    t24 = nc.dram_tensor("t24", [24, 1], f32, kind="ExternalInput")
    gb96 = nc.dram_tensor("gb96", [96, 1], f32, kind="ExternalInput")
    ident = nc.dram_tensor("ident", [128, 128], f32, kind="ExternalInput")

    out = nc.dram_tensor("out", [NPAD, FOUT], f32, kind="ExternalOutput")

    # Node permutation inside a superchunk: node = s*2560 + p*20 + q so that each
    # partition reads/writes one contiguous 10KB DRAM range per superchunk.
    # Identical views on input and output make the permutation self-consistent.
    xs_r = xs.rearrange("(s p q) m -> s p (q m)", p=128, q=QB)
    out_r = out.rearrange("(s p q) m -> s p (q m)", p=128, q=QB)

    with tile.TileContext(nc) as tc:
        with (
            tc.tile_pool(name="consts", bufs=1) as cpool,
            tc.tile_pool(name="xin", bufs=2) as xpool,
            tc.tile_pool(name="oout", bufs=2) as opool,
            tc.tile_pool(name="work", bufs=2) as wpool,
            tc.tile_pool(name="xt_ps", bufs=2, space="PSUM") as xtpp,
            tc.tile_pool(name="h_ps", bufs=1, space="PSUM") as hpp,
            tc.tile_pool(name="vrep_ps", bufs=1, space="PSUM") as vpp,
            tc.tile_pool(name="head_ps", bufs=2, space="PSUM") as hdpp,
            tc.tile_pool(name="outn_ps", bufs=2, space="PSUM") as onpp,
        ):
            # ---- load constants once ----
            # fp32r matmul operands must be *produced* as float32r (BIR
            # verifier); weights go through an f32 staging tile + cast copy.
            wf1_st = cpool.tile([FIN, FH], f32, name="wf1_st")
            nc.sync.dma_start(wf1_st[:], wf1[:, :])
            wf1_sb = cpool.tile([FIN, FH], f32r, name="wf1_sb")
            nc.vector.tensor_copy(wf1_sb[:], wf1_st[:])
            wf2r_st = cpool.tile([FH, 96], f32, name="wf2r_st")
            nc.sync.dma_start(wf2r_st[:], wf2r[:, :])
            wf2r_sb = cpool.tile([FH, 96], f32r, name="wf2r_sb")
            nc.vector.tensor_copy(wf2r_sb[:], wf2r_st[:])
            wout1_st = cpool.tile([FIN, FOUT], f32, name="wout1_st")
            nc.sync.dma_start(wout1_st[:], wout1[:, :])
            wout1_sb = cpool.tile([FIN, FOUT], f32r, name="wout1_sb")
            nc.vector.tensor_copy(wout1_sb[:], wout1_st[:])
            wout2p_st = cpool.tile([96, FOUT], f32, name="wout2p_st")
            nc.sync.dma_start(wout2p_st[:], wout2p[:, :])
            wout2p_sb = cpool.tile([96, FOUT], f32r, name="wout2p_sb")
            nc.vector.tensor_copy(wout2p_sb[:], wout2p_st[:])
            bf1_sb = cpool.tile([FH, 1], f32, name="bf1_sb")
            nc.sync.dma_start(bf1_sb[:], bf1[:, :])
            bout_sb = cpool.tile([FOUT, 1], f32, name="bout_sb")
            nc.sync.dma_start(bout_sb[:], boutc[:, :])
            a96_sb = cpool.tile([96, 1], f32, name="a96_sb")
            nc.sync.dma_start(a96_sb[:], a96[:, :])
            b96_sb = cpool.tile([96, 1], f32, name="b96_sb")
            nc.sync.dma_start(b96_sb[:], b96[:, :])
            t24_sb = cpool.tile([24, 1], f32, name="t24_sb")
            nc.sync.dma_start(t24_sb[:], t24[:, :])
            gb96_sb = cpool.tile([96, 1], f32, name="gb96_sb")
            nc.sync.dma_start(gb96_sb[:], gb96[:, :])
            ident_sb = cpool.tile([128, 128], f32, name="ident_sb")
            nc.sync.dma_start(ident_sb[:], ident[:, :])

            for s in range(NSUP):
                x_sb = xpool.tile([128, SUPER], f32, name="x_sb")
                nc.sync.dma_start(x_sb[:], xs_r[s])
                outn_sb = opool.tile([128, SUPER], f32, name="outn_sb")

                for ci in range(NCHUNK):
                    c0 = ci * CHUNK
                    # transpose 4 x-blocks: [node,fin] -> [fin,node]
                    xt_ps = xtpp.tile([128, CHUNK], f32, name="xt_ps")
                    for j in range(JB):
                        nc.tensor.transpose(
                            xt_ps[:, j * 128:(j + 1) * 128],
                            x_sb[:, c0 + j * 128:c0 + (j + 1) * 128],
                            ident_sb[:],
                        )
                    xt_sb = wpool.tile([128, CHUNK], f32r, name="xt_sb")
                    nc.vector.tensor_copy(xt_sb[:], xt_ps[:])

                    # h = relu(x @ Wf1 + bf1), feature-major [64, CHUNK]
                    h_ps = hpp.tile([FH, CHUNK], f32, name="h_ps")
                    nc.tensor.matmul(
                        h_ps[:], wf1_sb[:], xt_sb[:], start=True, stop=True,
                    )
                    h_sb = wpool.tile([FH, CHUNK], f32r, name="h_sb")
                    nc.scalar.activation(h_sb[:], h_ps[:], AF.Relu, bias=bf1_sb[:])

                    # vrep = h @ Wf2r : replicated pre-bias filtration rows [96, CHUNK]
                    vrep_ps = vpp.tile([96, CHUNK], f32, name="vrep_ps")
                    nc.tensor.matmul(
                        vrep_ps[:], wf2r_sb[:], h_sb[:], start=True, stop=True,
                    )

                    # act0 rows: affine a*v^ + b for all 96 rows
                    act_sb = wpool.tile([96, CHUNK], f32r, name="act_sb")
                    nc.vector.tensor_scalar(
                        act_sb[:], vrep_ps[:], a96_sb[:], b96_sb[:], OP.mult, OP.add
                    )
                    # triangle rows: max(min(u, t), 0)
                    nc.vector.tensor_scalar(
                        act_sb[0:24, :], act_sb[0:24, :], t24_sb[:], 0.0, OP.min, OP.max
                    )
                    # gaussian rows: exp(-(s^2) + gamma)
                    nc.vector.tensor_tensor(
                        act_sb[64:88, :], act_sb[64:88, :], act_sb[64:88, :], OP.mult
                    )
                    nc.scalar.activation(
                        act_sb[64:88, :], act_sb[64:88, :], AF.Exp,
                        bias=gb96_sb[64:88, :], scale=-1.0,
                    )

                    # head: out^T = Wout1^T x^T + Wout2p^T act0^T  [FOUT, CHUNK]
                    head_ps = hdpp.tile([FOUT, CHUNK], f32, name="head_ps")
                    nc.tensor.matmul(
                        head_ps[:], wout1_sb[:], xt_sb[:], start=True, stop=False,
                    )
                    nc.tensor.matmul(
                        head_ps[:], wout2p_sb[:], act_sb[:], start=False, stop=True,
                    )
                    outt_sb = wpool.tile([FOUT, CHUNK], f32, name="outt_sb")
                    nc.scalar.activation(outt_sb[:], head_ps[:], AF.Relu, bias=bout_sb[:])

                    # transpose back to node-major and stage into the superchunk tile
                    outn_ps = onpp.tile([128, CHUNK], f32, name="outn_ps")
                    for j in range(JB):
                        nc.tensor.transpose(
                            outn_ps[:, j * 128:(j + 1) * 128],
                            outt_sb[:, j * 128:(j + 1) * 128],
                            ident_sb[:],
                        )
                    nc.scalar.copy(outn_sb[:, c0:c0 + CHUNK], outn_ps[:])

                nc.sync.dma_start(out_r[s], outn_sb[:])

    nc.compile()
    _CACHE["nc"] = nc
    return nc


def _host_constants(inputs):
    """Fold params into the per-row device constants. Returns dict of arrays."""
    f32 = np.float32
    tri_t = np.asarray(inputs["tri_t"], f32)
    gauss_c = np.asarray(inputs["gauss_c"], f32)
    line_W = np.asarray(inputs["line_W"], f32)
    line_b = np.asarray(inputs["line_b"], f32)
    bf2 = np.asarray(inputs["bf2"], f32)
    Wf2 = np.asarray(inputs["Wf2"], f32)
    Wout = np.asarray(inputs["Wout"], f32)

    k_idx = np.repeat(np.arange(K), F)      # within-group row c -> k
    f_idx = np.tile(np.arange(F), K)        # within-group row c -> f

    cbar = (gauss_c[:, 0] + gauss_c[:, 1]) / 2.0
    gamma = 25.0 * (gauss_c[:, 0] + gauss_c[:, 1]) ** 2 - 50.0 * (
        gauss_c[:, 0] ** 2 + gauss_c[:, 1] ** 2
    )
    wlin = line_W[0] + line_W[1]

    # group base rows in the 96-row padded layout
    TRI0, LIN0, GAU0 = 0, 32, 64

    a96 = np.zeros(96, f32)
    b96 = np.zeros(96, f32)
    a96[TRI0:TRI0 + 24] = 2.0
    b96[TRI0:TRI0 + 24] = 2.0 * bf2[f_idx] - tri_t[k_idx]
    a96[LIN0:LIN0 + 24] = wlin[k_idx]
    b96[LIN0:LIN0 + 24] = wlin[k_idx] * bf2[f_idx] + line_b[k_idx]
    a96[GAU0:GAU0 + 24] = 10.0
    b96[GAU0:GAU0 + 24] = 10.0 * (bf2[f_idx] - cbar[k_idx])
    t24 = tri_t[k_idx].astype(f32)
    gb96 = np.zeros(96, f32)
    gb96[GAU0:GAU0 + 24] = gamma[k_idx]

    wf2r = np.zeros((FH, 96), f32)          # zero pad columns
    for g0 in (TRI0, LIN0, GAU0):
        wf2r[:, g0:g0 + 24] = np.tile(Wf2, (1, 3))

    # permuted Wout rows for the on-device act0 ordering (pad rows zero)
    wout2p = np.zeros((96, FOUT), f32)
    wout2p[TRI0:TRI0 + 24] = Wout[128 + f_idx * 9 + k_idx, :]
    wout2p[LIN0:LIN0 + 24] = Wout[128 + f_idx * 9 + 6 + k_idx, :]
    wout2p[GAU0:GAU0 + 24] = Wout[128 + f_idx * 9 + 3 + k_idx, :]

    return dict(
        a96=a96.reshape(96, 1), b96=b96.reshape(96, 1), t24=t24.reshape(24, 1),
        gb96=gb96.reshape(96, 1), wf2r=wf2r, wout2p=wout2p,
        wout1=Wout[:128, :].astype(f32),
        boutc=np.asarray(inputs["bout"], f32).reshape(FOUT, 1),
        wf1=np.asarray(inputs["Wf1"], f32),
        bf1=np.asarray(inputs["bf1"], f32).reshape(FH, 1),
        ident=np.eye(128, dtype=f32),
    )


def _host_g1(inputs):
    """Exact numpy replica of the dim-1 persistence branch (tiny)."""
    f32 = np.float32
    x = np.asarray(inputs["x"], f32)
    Wf1 = np.asarray(inputs["Wf1"], f32)
    bf1 = np.asarray(inputs["bf1"], f32)
    Wf2 = np.asarray(inputs["Wf2"], f32)
    bf2 = np.asarray(inputs["bf2"], f32)
    tri_t1 = np.asarray(inputs["tri_t1"], f32)
    gauss_c1 = np.asarray(inputs["gauss_c1"], f32)
    line_W1 = np.asarray(inputs["line_W1"], f32)
    line_b1 = np.asarray(inputs["line_b1"], f32)
    edge_index = np.asarray(inputs["edge_index"])
    edge_batch = np.asarray(inputs["edge_batch"])
    re = np.asarray(inputs["random_edges"])     # [B, F]

    fcols = np.arange(F)

    u = edge_index[0, re]                       # [B, F] node ids
    v = edge_index[1, re]
    nodes = np.unique(np.concatenate([u.ravel(), v.ravel()]))
    xn = x[nodes]
    fvn = np.maximum(xn @ Wf1 + bf1, 0.0) @ Wf2 + bf2   # [M, F] exact fp32
    iu = np.searchsorted(nodes, u)
    iv = np.searchsorted(nodes, v)
    vals = np.maximum(fvn[iu, fcols[None, :]], fvn[iv, fcols[None, :]])  # [B, F]

    edges_t = np.unique(re.ravel())
    pers = np.zeros((len(edges_t), F, 2), f32)
    pos = np.searchsorted(edges_t, re)          # [B, F]
    pers[pos, fcols[None, :], 1] = vals         # duplicate (e,f): last wins, like .at[].set

    mask = (pers != 0).any(axis=(1, 2))         # [M_e]
    p = pers[mask]                              # [M, F, 2]
    birth = p[..., 0:1]
    death = p[..., 1:2]
    tri = np.maximum(death - np.abs(tri_t1 - birth), 0.0)                    # [M,F,K]
    dif = p[..., None, :] - gauss_c1                                         # [M,F,K,2]
    gau = np.exp(-np.sum(dif * dif, -1) / (2.0 * SIGMA * SIGMA))
    lin = p @ line_W1 + line_b1
    act1 = np.concatenate([tri, gau, lin], axis=-1).reshape(-1, F * 3 * K).astype(f32)

    g1 = np.zeros((B, F * 3 * K), f32)
    np.add.at(g1, edge_batch[edges_t[mask]], act1)
    return g1


def kernel(**inputs):
    global LAST_RESULTS
    from concourse.bass_utils import run_bass_kernel_spmd

    f32 = np.float32
    x = np.asarray(inputs["x"], f32)

    consts = _host_constants(inputs)

    # shard + pad nodes
    shards = []
    for c in range(NCORES):
        xs = np.zeros((NPAD, FIN), f32)
        xs[:NSHARD] = x[c * NSHARD:(c + 1) * NSHARD]
        shards.append(xs)

    nc = _build_bass()

    in_maps = []
    for c in range(NCORES):
        m = {"xs": shards[c]}
        m.update({k: np.ascontiguousarray(v) for k, v in consts.items()})
        in_maps.append(m)

    trace = bool(int(os.environ.get("KERNEL_TRACE", "0")))
    res = run_bass_kernel_spmd(nc, in_maps, list(range(NCORES)), trace=trace)
    LAST_RESULTS = res

    out = np.empty((N, FOUT), f32)
    for c in range(NCORES):
        out[c * NSHARD:(c + 1) * NSHARD] = res.results[c]["out"][:NSHARD]

    g1 = _host_g1(inputs)
    return out, g1


# revision 14
# speedup vs baseline: 1.0402x; 1.0402x over previous
"""Trainium2 Bass kernel for nn_CoordfnTopologyLayer (TOGL-style coordinate-function
topology layer).

Contract: kernel(**inputs) takes the FULL unsharded inputs (as produced by the
problem's setup_inputs) and returns the FULL output tuple (out [N,128], g1 [B,72]).

Strategy (graph/data parallel per the sharding hint):
  - Shard the N=100000 nodes across 8 NeuronCores (12500 each, zero-padded to
    12800). All parameters are tiny and replicated.
  - Device computes, per node shard:
        h   = relu(x @ Wf1 + bf1)                  [n, 64]
        v^  = h @ Wf2  (bias folded into consts)   [n, 8]
        act0 = coord functions of v = v^+bf2       [n, 72]
        out = relu(x @ Wout[:128] + act0 @ Wout[128:] + bout)
    All matmuls run feature-major (features on partitions) with fp32r.
  - The dim-1 persistence output g1 [200, 72] depends only on the B*F = 1600
    randomly selected edges, i.e. <=3200 node filtration values. It is computed
    exactly on the host in numpy (~26 MFLOP).

Math identities used on device:
  triangle:  relu(v - |t - v|) = max(min(2v - t, t), 0)
  gaussian:  exp(-((v-c0)^2 + (v-c1)^2)/(2s^2)) = exp(-(10(v-cbar))^2 + gamma),
             s=0.1, cbar=(c0+c1)/2, gamma = 25(c0+c1)^2 - 50(c0^2+c1^2)
  line:      (W0+W1)*v + b
  The 8 -> 72 row replication (each filtration feeds 9 coord funcs) is folded
  into the second MLP weight: Wf2r = tile(Wf2, 9) so one matmul produces the
  replicated pre-activation rows directly.

Row order of the on-device act0 (and the correspondingly permuted Wout rows),
padded to 96 rows so every partition-sliced op starts 32-aligned:
  rows  0:24  triangle, k-major then f  (c = k*8+f);  24:32 zero pad
  rows 32:56  line;                                   56:64 zero pad
  rows 64:88  gaussian;                               88:96 zero pad
"""

import os
import sys

import numpy as np

if "/opt/trn_rl_repo" not in sys.path:
    sys.path.insert(0, "/opt/trn_rl_repo")

# ---- problem constants (hardcoded per contract) ----
N = 100000
E = 1600000
B = 200
F = 8
FIN = 128
FOUT = 128
FH = 64
K = 3
SIGMA = 0.1

NCORES = 8
NSHARD = N // NCORES          # 12500
NPAD = 12800                  # per-core padded node count: 5 superchunks * 2560
SUPER = 2560                  # nodes per superchunk (20 blocks of 128)
NSUP = NPAD // SUPER          # 5
QB = SUPER // 128             # 20 blocks per superchunk
CHUNK = 512                   # nodes per compute chunk (psum free-dim limit)
JB = CHUNK // 128             # 4 blocks per chunk
NCHUNK = SUPER // CHUNK       # 5 chunks per superchunk

_CACHE = {}
LAST_RESULTS = None           # BassKernelResults of the most recent run


def _build_bass():
    """Build + schedule the per-core Bass/Tile program. Cached per process."""
    if "nc" in _CACHE:
        return _CACHE["nc"]

    import concourse.bass as bass
    import concourse.tile as tile
    from concourse import bacc, mybir

    f32 = mybir.dt.float32
    f32r = mybir.dt.float32r
    AF = mybir.ActivationFunctionType
    OP = mybir.AluOpType

    nc = bacc.Bacc("TRN2", target_bir_lowering=False, debug=False)

    # ---- DRAM I/O ----
    xs = nc.dram_tensor("xs", [NPAD, FIN], f32, kind="ExternalInput")
    wf1 = nc.dram_tensor("wf1", [FIN, FH], f32, kind="ExternalInput")
    bf1 = nc.dram_tensor("bf1", [FH, 1], f32, kind="ExternalInput")
    wf2r = nc.dram_tensor("wf2r", [FH, 96], f32, kind="ExternalInput")
    wout1 = nc.dram_tensor("wout1", [FIN, FOUT], f32, kind="ExternalInput")
    wout2p = nc.dram_tensor("wout2p", [96, FOUT], f32, kind="ExternalInput")
    boutc = nc.dram_tensor("boutc", [FOUT, 1], f32, kind="ExternalInput")
    a96 = nc.dram_tensor("a96", [96, 1], f32, kind="ExternalInput")
    b96 = nc.dram_tensor("b96", [96, 1], f32, kind="ExternalInput")
    t24 = nc.dram_tensor("t24", [24, 1], f32, kind="ExternalInput")
    gb96 = nc.dram_tensor("gb96", [96, 1], f32, kind="ExternalInput")
    ident = nc.dram_tensor("ident", [128, 128], f32, kind="ExternalInput")

    out = nc.dram_tensor("out", [NPAD, FOUT], f32, kind="ExternalOutput")

    # Node permutation inside a superchunk: node = s*2560 + p*20 + q so that each
    # partition reads/writes one contiguous 10KB DRAM range per superchunk.
    # Identical views on input and output make the permutation self-consistent.
    xs_r = xs.rearrange("(s p q) m -> s p (q m)", p=128, q=QB)
    out_r = out.rearrange("(s p q) m -> s p (q m)", p=128, q=QB)

    with tile.TileContext(nc) as tc:
        with (
            tc.tile_pool(name="consts", bufs=1) as cpool,
            tc.tile_pool(name="xin", bufs=2) as xpool,
            tc.tile_pool(name="oout", bufs=2) as opool,
            tc.tile_pool(name="work", bufs=3) as wpool,
            tc.tile_pool(name="xt_ps", bufs=2, space="PSUM") as xtpp,
            tc.tile_pool(name="h_ps", bufs=2, space="PSUM") as hpp,
            tc.tile_pool(name="vrep_ps", bufs=1, space="PSUM") as vpp,
            tc.tile_pool(name="head_ps", bufs=2, space="PSUM") as hdpp,
            tc.tile_pool(name="outn_ps", bufs=1, space="PSUM") as onpp,
        ):
            # ---- load constants once ----
            # fp32r matmul operands must be *produced* as float32r (BIR
            # verifier); weights go through an f32 staging tile + cast copy.
            wf1_st = cpool.tile([FIN, FH], f32, name="wf1_st")
            nc.sync.dma_start(wf1_st[:], wf1[:, :])
            wf1_sb = cpool.tile([FIN, FH], f32r, name="wf1_sb")
            nc.vector.tensor_copy(wf1_sb[:], wf1_st[:])
            wf2r_st = cpool.tile([FH, 96], f32, name="wf2r_st")
            nc.sync.dma_start(wf2r_st[:], wf2r[:, :])
            wf2r_sb = cpool.tile([FH, 96], f32r, name="wf2r_sb")
            nc.vector.tensor_copy(wf2r_sb[:], wf2r_st[:])
            wout1_st = cpool.tile([FIN, FOUT], f32, name="wout1_st")
            nc.sync.dma_start(wout1_st[:], wout1[:, :])
            wout1_sb = cpool.tile([FIN, FOUT], f32r, name="wout1_sb")
            nc.vector.tensor_copy(wout1_sb[:], wout1_st[:])
            wout2p_st = cpool.tile([96, FOUT], f32, name="wout2p_st")
            nc.sync.dma_start(wout2p_st[:], wout2p[:, :])
            wout2p_sb = cpool.tile([96, FOUT], f32r, name="wout2p_sb")
            nc.vector.tensor_copy(wout2p_sb[:], wout2p_st[:])
            bf1_sb = cpool.tile([FH, 1], f32, name="bf1_sb")
            nc.sync.dma_start(bf1_sb[:], bf1[:, :])
            bout_sb = cpool.tile([FOUT, 1], f32, name="bout_sb")
            nc.sync.dma_start(bout_sb[:], boutc[:, :])
            a96_sb = cpool.tile([96, 1], f32, name="a96_sb")
            nc.sync.dma_start(a96_sb[:], a96[:, :])
            b96_sb = cpool.tile([96, 1], f32, name="b96_sb")
            nc.sync.dma_start(b96_sb[:], b96[:, :])
            t24_sb = cpool.tile([24, 1], f32, name="t24_sb")
            nc.sync.dma_start(t24_sb[:], t24[:, :])
            gb96_sb = cpool.tile([96, 1], f32, name="gb96_sb")
            nc.sync.dma_start(gb96_sb[:], gb96[:, :])
            ident_sb = cpool.tile([128, 128], f32, name="ident_sb")
            nc.sync.dma_start(ident_sb[:], ident[:, :])

            for s in range(NSUP):
                x_sb = xpool.tile([128, SUPER], f32, name="x_sb")
                nc.sync.dma_start(x_sb[:], xs_r[s])
                outn_sb = opool.tile([128, SUPER], f32, name="outn_sb")

                for ci in range(NCHUNK):
                    c0 = ci * CHUNK
                    # transpose 4 x-blocks: [node,fin] -> [fin,node]
                    xt_ps = xtpp.tile([128, CHUNK], f32, name="xt_ps")
                    for j in range(JB):
                        nc.tensor.transpose(
                            xt_ps[:, j * 128:(j + 1) * 128],
                            x_sb[:, c0 + j * 128:c0 + (j + 1) * 128],
                            ident_sb[:],
                        )
                    xt_sb = wpool.tile([128, CHUNK], f32r, name="xt_sb")
                    nc.vector.tensor_copy(xt_sb[:], xt_ps[:])

                    # h = relu(x @ Wf1 + bf1), feature-major [64, CHUNK]
                    h_ps = hpp.tile([FH, CHUNK], f32, name="h_ps")
                    nc.tensor.matmul(
                        h_ps[:], wf1_sb[:], xt_sb[:], start=True, stop=True,
                    )
                    h_sb = wpool.tile([FH, CHUNK], f32r, name="h_sb")
                    nc.vector.tensor_scalar(
                        h_sb[:], h_ps[:], bf1_sb[:], 0.0, OP.add, OP.max
                    )

                    # vrep = h @ Wf2r : replicated pre-bias filtration rows [96, CHUNK]
                    vrep_ps = vpp.tile([96, CHUNK], f32, name="vrep_ps")
                    nc.tensor.matmul(
                        vrep_ps[:], wf2r_sb[:], h_sb[:], start=True, stop=True,
                    )

                    # act0 rows: affine a*v^ + b for all 96 rows
                    act_sb = wpool.tile([96, CHUNK], f32r, name="act_sb")
                    nc.vector.tensor_scalar(
                        act_sb[:], vrep_ps[:], a96_sb[:], b96_sb[:], OP.mult, OP.add
                    )
                    # triangle rows: max(min(u, t), 0)
                    nc.gpsimd.tensor_scalar(
                        act_sb[0:24, :], act_sb[0:24, :], t24_sb[:], 0.0, OP.min, OP.max
                    )
                    # gaussian rows: exp(-(s^2) + gamma)
                    nc.gpsimd.tensor_tensor(
                        act_sb[64:88, :], act_sb[64:88, :], act_sb[64:88, :], OP.mult
                    )
                    nc.scalar.activation(
                        act_sb[64:88, :], act_sb[64:88, :], AF.Exp,
                        bias=gb96_sb[64:88, :], scale=-1.0,
                    )

                    # head: out^T = Wout1^T x^T + Wout2p^T act0^T  [FOUT, CHUNK]
                    head_ps = hdpp.tile([FOUT, CHUNK], f32, name="head_ps")
                    nc.tensor.matmul(
                        head_ps[:], wout1_sb[:], xt_sb[:], start=True, stop=False,
                    )
                    nc.tensor.matmul(
                        head_ps[:], wout2p_sb[:], act_sb[:], start=False, stop=True,
                    )
                    outt_sb = wpool.tile([FOUT, CHUNK], f32, name="outt_sb")
                    nc.scalar.activation(outt_sb[:], head_ps[:], AF.Relu, bias=bout_sb[:])

                    # transpose back to node-major and stage into the superchunk tile
                    outn_ps = onpp.tile([128, CHUNK], f32, name="outn_ps")
                    for j in range(JB):
                        nc.tensor.transpose(
                            outn_ps[:, j * 128:(j + 1) * 128],
                            outt_sb[:, j * 128:(j + 1) * 128],
                            ident_sb[:],
                        )
                    nc.scalar.copy(outn_sb[:, c0:c0 + CHUNK], outn_ps[:])

                nc.sync.dma_start(out_r[s], outn_sb[:])

    nc.compile()
    _CACHE["nc"] = nc
    return nc


def _host_constants(inputs):
    """Fold params into the per-row device constants. Returns dict of arrays."""
    f32 = np.float32
    tri_t = np.asarray(inputs["tri_t"], f32)
    gauss_c = np.asarray(inputs["gauss_c"], f32)
    line_W = np.asarray(inputs["line_W"], f32)
    line_b = np.asarray(inputs["line_b"], f32)
    bf2 = np.asarray(inputs["bf2"], f32)
    Wf2 = np.asarray(inputs["Wf2"], f32)
    Wout = np.asarray(inputs["Wout"], f32)

    k_idx = np.repeat(np.arange(K), F)      # within-group row c -> k
    f_idx = np.tile(np.arange(F), K)        # within-group row c -> f

    cbar = (gauss_c[:, 0] + gauss_c[:, 1]) / 2.0
    gamma = 25.0 * (gauss_c[:, 0] + gauss_c[:, 1]) ** 2 - 50.0 * (
        gauss_c[:, 0] ** 2 + gauss_c[:, 1] ** 2
    )
    wlin = line_W[0] + line_W[1]

    # group base rows in the 96-row padded layout
    TRI0, LIN0, GAU0 = 0, 32, 64

    a96 = np.zeros(96, f32)
    b96 = np.zeros(96, f32)
    a96[TRI0:TRI0 + 24] = 2.0
    b96[TRI0:TRI0 + 24] = 2.0 * bf2[f_idx] - tri_t[k_idx]
    a96[LIN0:LIN0 + 24] = wlin[k_idx]
    b96[LIN0:LIN0 + 24] = wlin[k_idx] * bf2[f_idx] + line_b[k_idx]
    a96[GAU0:GAU0 + 24] = 10.0
    b96[GAU0:GAU0 + 24] = 10.0 * (bf2[f_idx] - cbar[k_idx])
    t24 = tri_t[k_idx].astype(f32)
    gb96 = np.zeros(96, f32)
    gb96[GAU0:GAU0 + 24] = gamma[k_idx]

    wf2r = np.zeros((FH, 96), f32)          # zero pad columns
    for g0 in (TRI0, LIN0, GAU0):
        wf2r[:, g0:g0 + 24] = np.tile(Wf2, (1, 3))

    # permuted Wout rows for the on-device act0 ordering (pad rows zero)
    wout2p = np.zeros((96, FOUT), f32)
    wout2p[TRI0:TRI0 + 24] = Wout[128 + f_idx * 9 + k_idx, :]
    wout2p[LIN0:LIN0 + 24] = Wout[128 + f_idx * 9 + 6 + k_idx, :]
    wout2p[GAU0:GAU0 + 24] = Wout[128 + f_idx * 9 + 3 + k_idx, :]

    return dict(
        a96=a96.reshape(96, 1), b96=b96.reshape(96, 1), t24=t24.reshape(24, 1),
        gb96=gb96.reshape(96, 1), wf2r=wf2r, wout2p=wout2p,
        wout1=Wout[:128, :].astype(f32),
        boutc=np.asarray(inputs["bout"], f32).reshape(FOUT, 1),
        wf1=np.asarray(inputs["Wf1"], f32),
        bf1=np.asarray(inputs["bf1"], f32).reshape(FH, 1),
        ident=np.eye(128, dtype=f32),
    )


def _host_g1(inputs):
    """Exact numpy replica of the dim-1 persistence branch (tiny)."""
    f32 = np.float32
    x = np.asarray(inputs["x"], f32)
    Wf1 = np.asarray(inputs["Wf1"], f32)
    bf1 = np.asarray(inputs["bf1"], f32)
    Wf2 = np.asarray(inputs["Wf2"], f32)
    bf2 = np.asarray(inputs["bf2"], f32)
    tri_t1 = np.asarray(inputs["tri_t1"], f32)
    gauss_c1 = np.asarray(inputs["gauss_c1"], f32)
    line_W1 = np.asarray(inputs["line_W1"], f32)
    line_b1 = np.asarray(inputs["line_b1"], f32)
    edge_index = np.asarray(inputs["edge_index"])
    edge_batch = np.asarray(inputs["edge_batch"])
    re = np.asarray(inputs["random_edges"])     # [B, F]

    fcols = np.arange(F)

    u = edge_index[0, re]                       # [B, F] node ids
    v = edge_index[1, re]
    nodes = np.unique(np.concatenate([u.ravel(), v.ravel()]))
    xn = x[nodes]
    fvn = np.maximum(xn @ Wf1 + bf1, 0.0) @ Wf2 + bf2   # [M, F] exact fp32
    iu = np.searchsorted(nodes, u)
    iv = np.searchsorted(nodes, v)
    vals = np.maximum(fvn[iu, fcols[None, :]], fvn[iv, fcols[None, :]])  # [B, F]

    edges_t = np.unique(re.ravel())
    pers = np.zeros((len(edges_t), F, 2), f32)
    pos = np.searchsorted(edges_t, re)          # [B, F]
    pers[pos, fcols[None, :], 1] = vals         # duplicate (e,f): last wins, like .at[].set

    mask = (pers != 0).any(axis=(1, 2))         # [M_e]
    p = pers[mask]                              # [M, F, 2]
    birth = p[..., 0:1]
    death = p[..., 1:2]
    tri = np.maximum(death - np.abs(tri_t1 - birth), 0.0)                    # [M,F,K]
    dif = p[..., None, :] - gauss_c1                                         # [M,F,K,2]
    gau = np.exp(-np.sum(dif * dif, -1) / (2.0 * SIGMA * SIGMA))
    lin = p @ line_W1 + line_b1
    act1 = np.concatenate([tri, gau, lin], axis=-1).reshape(-1, F * 3 * K).astype(f32)

    g1 = np.zeros((B, F * 3 * K), f32)
    np.add.at(g1, edge_batch[edges_t[mask]], act1)
    return g1


def kernel(**inputs):
    global LAST_RESULTS
    from concourse.bass_utils import run_bass_kernel_spmd

    f32 = np.float32
    x = np.asarray(inputs["x"], f32)

    consts = _host_constants(inputs)

    # shard + pad nodes
    shards = []
    for c in range(NCORES):
        xs = np.zeros((NPAD, FIN), f32)
        xs[:NSHARD] = x[c * NSHARD:(c + 1) * NSHARD]
        shards.append(xs)

    nc = _build_bass()

    in_maps = []
    for c in range(NCORES):
        m = {"xs": shards[c]}
        m.update({k: np.ascontiguousarray(v) for k, v in consts.items()})
        in_maps.append(m)

    trace = bool(int(os.environ.get("KERNEL_TRACE", "0")))
    res = run_bass_kernel_spmd(nc, in_maps, list(range(NCORES)), trace=trace)
    LAST_RESULTS = res

    out = np.empty((N, FOUT), f32)
    for c in range(NCORES):
        out[c * NSHARD:(c + 1) * NSHARD] = res.results[c]["out"][:NSHARD]

    g1 = _host_g1(inputs)
    return out, g1
